# revision 3
# baseline (speedup 1.0000x reference)
"""Trainium2 Bass kernel for ClimateConditionedGAT (GATConv + one-hot prior gate).

Strategy (8 NeuronCores, SPMD single NEFF):
  - Nodes sharded by destination: core c owns dst nodes [c*6250, (c+1)*6250).
  - Phase 1 (replicated on every core): T[n] = [h(256) | a_src(4) | a_dst(4)]
    where h = x @ W_gat, a_* = per-head dots; stored in a DRAM table with
    1280B rows (dma_gather needs 256B-multiple rows/strides), split into two
    25001-row source buckets (dma_gather indices are int16) each with a
    sentinel row (a_src = -1e30 => exp -> 0) used for padding edges.
    A[n] = a_dst[n] (flat [N,4], gathered as [N/16, 64] rows by dst//16).
  - Phase 2: edges partitioned by (dst-window of 128, src bucket), sorted on
    host; per <=8-block group one dma_gather of T rows by src and one of A
    rows by dst//16; batched DVE/ACT ops compute ex = exp(leaky(asrc+adst));
    per 128-edge block a selection matrix S[e,d] = (dst_local==d) built by
    iota-compare feeds a PE matmul accumulating [sum ex*h | sum ex] into a
    per-window PSUM tile; window epilogue normalizes, averages heads, adds
    bias, computes prior = kg_onehot @ W_prior and gates the two.
All float math on device; host does integer edge partitioning/packing only.
"""
import sys
import numpy as np

if "/opt/trn_rl_repo" not in sys.path:
    sys.path.insert(0, "/opt/trn_rl_repo")

from contextlib import ExitStack

import concourse.bass as bass
import concourse.bacc as bacc
import concourse.mybir as mybir
import concourse.tile as tile
from concourse.bass_utils import run_bass_kernel_spmd

P = 128
N = 50000
DIN = 128
HEADS = 4
F = 64
HF = HEADS * F          # 256
NCORES = 8
SH = N // NCORES        # 6250 dst nodes per core
ROW = 320               # T row in f32 elems (1280 B)
BUCK_N = 25000          # real nodes per source bucket
BROWS = BUCK_N + 1      # bucket rows incl sentinel
TROWS = 2 * BROWS       # 50002
SENT = BUCK_N           # bucket-local sentinel index
WIN = P                 # dst window size
NWIN = (SH + WIN - 1) // WIN    # 49 windows per core
GMAX = 8                # max 128-edge blocks per dma_gather (1024 idx limit)
NEG_SLOPE = 0.2
dt = mybir.dt
Alu = mybir.AluOpType
Act = mybir.ActivationFunctionType


# ----------------------------------------------------------------------------
# host-side edge preparation (integer/layout work only)
# ----------------------------------------------------------------------------

def _pack_idx16(idx):
    """Pack int array -> [128, ceil(n/16)] int16 (wrap 16, replicate x8)."""
    n = len(idx)
    cols = (n + 15) // 16
    a = np.zeros((16, cols), np.int16)
    a[np.arange(n) % 16, np.arange(n) // 16] = idx.astype(np.int16)
    return np.tile(a, (8, 1))


def _prep_edges(edge_index):
    src = np.asarray(edge_index[0], dtype=np.int64)
    dst = np.asarray(edge_index[1], dtype=np.int64)
    loops = np.arange(N, dtype=np.int64)
    src = np.concatenate([src, loops])
    dst = np.concatenate([dst, loops])

    core = dst // SH
    w = (dst % SH) // WIN
    b = (src >= BUCK_N).astype(np.int64)

    # group id per edge: (core, w, b)
    gid = (core * NWIN + w) * 2 + b
    ngroups = NCORES * NWIN * 2
    counts = np.bincount(gid, minlength=ngroups).reshape(NCORES, NWIN, 2)

    # uniform block counts across cores (shared NEFF)
    B_wb = np.ceil(counts.max(axis=0) / P).astype(np.int64)      # [NWIN, 2]

    # chunks of <= GMAX blocks per gather
    chunks = []  # list of (w, b, nblk, blk0) in emission order; blk0 = global block idx
    blk0 = 0
    for wi in range(NWIN):
        for bi in range(2):
            rem = int(B_wb[wi, bi])
            while rem > 0:
                g = min(rem, GMAX)
                chunks.append((wi, bi, g, blk0))
                blk0 += g
                rem -= g
    cblk = blk0                      # total blocks per core
    slots = cblk * P

    order = np.argsort(gid, kind="stable")
    src_s, dst_s, gid_s = src[order], dst[order], gid[order]
    # start offset of each (c,w,b) group inside the sorted edge array
    grp_starts = np.zeros(ngroups + 1, np.int64)
    np.cumsum(np.bincount(gid_s, minlength=ngroups), out=grp_starts[1:])

    # slot offset of each (w,b) group inside the padded per-core layout
    slot_off = np.zeros((NWIN, 2), np.int64)
    off = 0
    for wi in range(NWIN):
        for bi in range(2):
            slot_off[wi, bi] = off
            off += int(B_wb[wi, bi]) * P

    midx = np.full((NCORES, slots), SENT, np.int64)   # bucket-local src row
    tidx = np.zeros((NCORES, slots), np.int64)        # dst // 16 (global)
    dmod = np.zeros((NCORES, slots), np.float32)      # dst % 16
    dstf = np.zeros((NCORES, slots), np.float32)      # dst local to window

    for c in range(NCORES):
        for wi in range(NWIN):
            for bi in range(2):
                g = (c * NWIN + wi) * 2 + bi
                s0, s1 = grp_starts[g], grp_starts[g + 1]
                n = s1 - s0
                if n == 0:
                    continue
                o = slot_off[wi, bi]
                es, ed = src_s[s0:s1], dst_s[s0:s1]
                midx[c, o:o + n] = es - bi * BUCK_N
                tidx[c, o:o + n] = ed // 16
                dmod[c, o:o + n] = (ed % 16).astype(np.float32)
                dstf[c, o:o + n] = (ed - (c * SH + wi * WIN)).astype(np.float32)

    midx_p = np.stack([_pack_idx16(midx[c]) for c in range(NCORES)])
    tidx_p = np.stack([_pack_idx16(tidx[c]) for c in range(NCORES)])
    # column-per-block layout for dmod/dstf: slot e -> [e % 128, e // 128]
    dmod_c = dmod.reshape(NCORES, cblk, P).transpose(0, 2, 1).copy()
    dstf_c = dstf.reshape(NCORES, cblk, P).transpose(0, 2, 1).copy()
    return chunks, B_wb, cblk, midx_p, tidx_p, dmod_c, dstf_c


# ----------------------------------------------------------------------------
# device program
# ----------------------------------------------------------------------------

def _build_nc(chunks, cblk):
    nc = bacc.Bacc("TRN2", target_bir_lowering=False, debug=False)

    t_x = nc.dram_tensor("x", [N, DIN], dt.float32, kind="ExternalInput")
    t_wg = nc.dram_tensor("wg", [DIN, HF], dt.float32, kind="ExternalInput")
    t_asb = nc.dram_tensor("attsrc_b", [P, HF], dt.float32, kind="ExternalInput")
    t_adb = nc.dram_tensor("attdst_b", [P, HF], dt.float32, kind="ExternalInput")
    t_bg = nc.dram_tensor("biasg_b", [P, F], dt.float32, kind="ExternalInput")
    t_bp = nc.dram_tensor("bprior_b", [P, F], dt.float32, kind="ExternalInput")
    t_gate = nc.dram_tensor("gate_b", [P, 1], dt.float32, kind="ExternalInput")
    t_wp = nc.dram_tensor("wp", [32, F], dt.float32, kind="ExternalInput")
    t_kg = nc.dram_tensor("kg", [SH, 32], dt.float32, kind="ExternalInput")
    t_iota = nc.dram_tensor("iota_g", [P, GMAX * P], dt.float32, kind="ExternalInput")
    t_io16 = nc.dram_tensor("iota16_g", [P, GMAX * 16], dt.float32, kind="ExternalInput")
    mcols = (cblk * P) // 16
    t_midx = nc.dram_tensor("midx", [P, mcols], dt.int16, kind="ExternalInput")
    t_tidx = nc.dram_tensor("tidx", [P, mcols], dt.int16, kind="ExternalInput")
    t_dmod = nc.dram_tensor("dmod", [P, cblk], dt.float32, kind="ExternalInput")
    t_dstf = nc.dram_tensor("dstf", [P, cblk], dt.float32, kind="ExternalInput")
    t_out = nc.dram_tensor("out", [SH, F], dt.float32, kind="ExternalOutput")

    t_T = nc.dram_tensor("tbl", [TROWS, ROW], dt.float32)
    t_A = nc.dram_tensor("adst", [N + 48, HEADS], dt.float32)
    a_view = bass.AP(t_A[:, :].tensor, 0, [[64, N // 16], [1, 64]])

    from concourse.masks import make_identity

    with tile.TileContext(nc) as tc:
        with ExitStack() as octx:
            cp = octx.enter_context(tc.tile_pool(name="const", bufs=1))

            ident = cp.tile([P, P], dt.float32)
            make_identity(nc, ident[:])
            iota_sb = cp.tile([P, GMAX * P], dt.float32)
            nc.sync.dma_start(iota_sb[:], t_iota[:])
            io16_sb = cp.tile([P, GMAX * 16], dt.float32)
            nc.sync.dma_start(io16_sb[:], t_io16[:])
            wp_sb = cp.tile([32, F], dt.float32)
            nc.sync.dma_start(wp_sb[:], t_wp[:])
            asb = cp.tile([P, HF], dt.float32)
            nc.sync.dma_start(asb[:], t_asb[:])
            adb = cp.tile([P, HF], dt.float32)
            nc.sync.dma_start(adb[:], t_adb[:])
            bg_sb = cp.tile([P, F], dt.float32)
            nc.sync.dma_start(bg_sb[:], t_bg[:])
            bp_sb = cp.tile([P, F], dt.float32)
            nc.sync.dma_start(bp_sb[:], t_bp[:])
            gate_sb = cp.tile([P, 1], dt.float32)
            nc.sync.dma_start(gate_sb[:], t_gate[:])
            g1m_sb = cp.tile([P, 1], dt.float32)
            nc.vector.tensor_scalar(g1m_sb[:], gate_sb[:], -1.0, 1.0,
                                    op0=Alu.mult, op1=Alu.add)
            midx_sb = cp.tile([P, mcols], dt.int16)
            nc.sync.dma_start(midx_sb[:], t_midx[:])
            tidx_sb = cp.tile([P, mcols], dt.int16)
            nc.sync.dma_start(tidx_sb[:], t_tidx[:])
            dmod_sb = cp.tile([P, cblk], dt.float32)
            nc.sync.dma_start(dmod_sb[:], t_dmod[:])
            dstf_sb = cp.tile([P, cblk], dt.float32)
            nc.sync.dma_start(dstf_sb[:], t_dstf[:])

            # W_ext = [W_gat | wsrc(4) | wdst(4)] where wsrc_j = W[:,j]·att_src[j]
            wext = cp.tile([P, HF + 8], dt.float32)
            nc.sync.dma_start(wext[:, 0:HF], t_wg[:])
            with tc.tile_pool(name="wtmp", bufs=2) as wtp:
                for j in range(HEADS):
                    for k, att in enumerate((asb, adb)):
                        tmp = wtp.tile([P, F], dt.float32, tag="wtmp")
                        nc.vector.tensor_tensor(
                            out=tmp[:], in0=wext[:, j * F:(j + 1) * F],
                            in1=att[:, j * F:(j + 1) * F], op=Alu.mult)
                        nc.vector.tensor_reduce(
                            out=wext[:, HF + 4 * k + j:HF + 4 * k + j + 1],
                            in_=tmp[:], axis=mybir.AxisListType.X, op=Alu.add)

            # ---------------- phase 1: build T and A ----------------
            with tc.tile_pool(name="p1sb", bufs=3) as p1, \
                 tc.tile_pool(name="p1ps", bufs=2, space="PSUM") as pp1, \
                 tc.tile_pool(name="p1ac", bufs=2) as p1a:
                ACH = 32  # blocks of a_dst staged per A-write DMA
                for bkt in range(2):
                    nfull = BUCK_N // P           # 195 full blocks
                    nblk = (BUCK_N + P - 1) // P  # incl partial
                    ach_tile = None
                    ach_base = 0
                    for bi in range(nblk):
                        ns = bkt * BUCK_N + bi * P
                        rs = bkt * BROWS + bi * P
                        nr = min(P, BUCK_N - bi * P)
                        full = nr == P
                        ci = bi % ACH
                        if full and ci == 0:
                            ach_tile = p1a.tile([P, ACH, HEADS], dt.float32,
                                                tag="ach")
                            ach_base = ns
                        xb = p1.tile([P, DIN], dt.float32, tag="xb")
                        if nr < P:
                            nc.gpsimd.memset(xb[:], 0.0)
                        nc.sync.dma_start(xb[:nr, :], t_x[ns:ns + nr, :])
                        xt_ps = pp1.tile([P, P], dt.float32, space="PSUM",
                                         tag="xt")
                        nc.tensor.transpose(out=xt_ps[:], in_=xb[:],
                                            identity=ident[:])
                        xt_sb = p1.tile([P, P], dt.float32, tag="xts")
                        nc.scalar.copy(xt_sb[:], xt_ps[:])
                        h_ps = pp1.tile([P, HF + 8], dt.float32, space="PSUM",
                                        tag="hps")
                        nc.tensor.matmul(h_ps[:], lhsT=xt_sb[:], rhs=wext[:],
                                         start=True, stop=True)
                        row = p1.tile([P, HF + 8], dt.float32, tag="row")
                        nc.scalar.copy(row[:], h_ps[:])
                        nc.sync.dma_start(t_T[rs:rs + nr, 0:HF + 8], row[:nr, :])
                        if full:
                            nc.scalar.copy(ach_tile[:, ci, :],
                                           h_ps[:, HF + 4:HF + 8])
                            if ci == ACH - 1 or bi == nfull - 1:
                                nb = ci + 1
                                dst_ap = t_A[ach_base:ach_base + nb * P, :] \
                                    .rearrange("(b p) c -> p b c", p=P)
                                nc.sync.dma_start(dst_ap, ach_tile[:, 0:nb, :])
                        else:
                            # partial tail block: write its a_dst rows directly
                            nc.sync.dma_start(t_A[ns:ns + nr, :],
                                              row[:nr, HF + 4:HF + 8])
                # sentinel rows
                sent = p1.tile([1, ROW], dt.float32, tag="sent")
                nc.gpsimd.memset(sent[:], 0.0)
                nc.gpsimd.memset(sent[:, HF:HF + 4], -1e30)
                nc.sync.dma_start(t_T[SENT:SENT + 1, :], sent[:])
                nc.sync.dma_start(t_T[BROWS + SENT:BROWS + SENT + 1, :], sent[:])

            # ---------------- phase 2: gather / scatter ----------------
            with tc.tile_pool(name="g1p", bufs=2) as g1p, \
                 tc.tile_pool(name="g2p", bufs=2) as g2p, \
                 tc.tile_pool(name="wk", bufs=3) as wk, \
                 tc.tile_pool(name="sp", bufs=3) as sp, \
                 tc.tile_pool(name="accp", bufs=2, space="PSUM") as accp, \
                 tc.tile_pool(name="prp", bufs=2, space="PSUM") as prp, \
                 tc.tile_pool(name="fin", bufs=3) as fin:

                acc_ps = None
                cur_w = -1
                # chunk index ranges per window for start/stop flags
                win_first = {}
                win_last = {}
                for ci, (wi, bi, g, blk0) in enumerate(chunks):
                    win_first.setdefault(wi, ci)
                    win_last[wi] = ci

                for ci, (wi, bi, g, blk0) in enumerate(chunks):
                    if win_first[wi] == ci:
                        acc_ps = accp.tile([P, HF + 4], dt.float32, space="PSUM",
                                           tag="acc")
                        cur_w = wi
                    nidx = g * P
                    ic0 = (blk0 * P) // 16
                    icn = nidx // 16
                    g1 = g1p.tile([P, GMAX, ROW], dt.float32, tag="g1")
                    nc.gpsimd.dma_gather(
                        g1[:, 0:g, :], t_T[bi * BROWS:(bi + 1) * BROWS, :],
                        midx_sb[:, ic0:ic0 + icn], nidx, nidx, ROW)
                    g2 = g2p.tile([P, GMAX, 64], dt.float32, tag="g2")
                    nc.gpsimd.dma_gather(
                        g2[:, 0:g, :], a_view, tidx_sb[:, ic0:ic0 + icn],
                        nidx, nidx, 64)

                    def v3(t, d1, s1, d2, s2):
                        return bass.AP(t.tensor, t.offset,
                                       [t.ap[0], [s1, d1], [s2, d2]])

                    def v4(t, d1, s1, d2, s2, d3, s3):
                        return bass.AP(t.tensor, t.offset,
                                       [t.ap[0], [s1, d1], [s2, d2], [s3, d3]])

                    g1a = g1[:, 0:g, :]
                    g2a = g2[:, 0:g, :]
                    # mask[e, m] = (dst%16 == m)   [P, g, 16]
                    mask = wk.tile([P, GMAX * 16], dt.float32, tag="mask")
                    nc.vector.tensor_tensor(
                        out=v3(mask[:], g, 16, 16, 1),
                        in0=v3(dmod_sb[:, blk0:blk0 + g], g, 1, 16, 0),
                        in1=v3(io16_sb[:, 0:g * 16], g, 16, 16, 1),
                        op=Alu.is_equal)
                    # adst_sel[e, j] = sum_m mask[e,m] * A_g[e, m*4+j]
                    selt = wk.tile([P, GMAX * 64], dt.float32, tag="selt")
                    nc.vector.tensor_tensor(
                        out=v4(selt[:], g, 64, 16, 4, 4, 1),
                        in0=v4(mask[:], g, 16, 16, 1, 4, 0),
                        in1=v4(g2a, g, 64, 16, 4, 4, 1),
                        op=Alu.mult)
                    adst = wk.tile([P, GMAX * 4], dt.float32, tag="adst")
                    nc.vector.tensor_reduce(
                        out=v3(adst[:], g, 4, 4, 1),
                        in_=v4(selt[:], g, 64, 4, 1, 16, 4),
                        axis=mybir.AxisListType.X, op=Alu.add)
                    # alpha = leaky(asrc + adst); ex = exp(alpha)
                    alpha = wk.tile([P, GMAX * 4], dt.float32, tag="alpha")
                    nc.vector.tensor_tensor(
                        out=v3(alpha[:], g, 4, 4, 1),
                        in0=v3(bass.AP(g1a.tensor, g1a.offset + HF,
                                       [g1a.ap[0]]), g, ROW, 4, 1),
                        in1=v3(adst[:], g, 4, 4, 1), op=Alu.add)
                    nc.vector.scalar_tensor_tensor(
                        out=alpha[:, 0:g * 4], in0=alpha[:, 0:g * 4],
                        scalar=NEG_SLOPE, in1=alpha[:, 0:g * 4],
                        op0=Alu.mult, op1=Alu.max)
                    ex = wk.tile([P, GMAX * 4], dt.float32, tag="ex")
                    nc.scalar.activation(ex[:, 0:g * 4], alpha[:, 0:g * 4], Act.Exp)
                    # S[e, d] = (dstf == d)   [P, g, 128]
                    S = sp.tile([P, GMAX * P], dt.float32, tag="S")
                    nc.vector.tensor_tensor(
                        out=v3(S[:], g, P, P, 1),
                        in0=v3(dstf_sb[:, blk0:blk0 + g], g, 1, P, 0),
                        in1=v3(iota_sb[:, 0:g * P], g, P, P, 1),
                        op=Alu.is_equal)
                    # rhs = [ex*h (256) | ex (4)]   [P, g, 260]
                    rhs = wk.tile([P, GMAX * (HF + 4)], dt.float32, tag="rhs")

                    def hview(t, base_step, hoff, nh, hstep):
                        return bass.AP(t.tensor, t.offset + hoff,
                                       [t.ap[0], [base_step, g], [hstep * F, nh],
                                        [1, F]])

                    def exview(hoff):
                        return bass.AP(ex[:].tensor, ex[:].offset + hoff,
                                       [ex[:].ap[0], [4, g], [1, 2], [0, F]])

                    nc.vector.tensor_tensor(
                        out=hview(rhs[:], HF + 4, 0, 2, 1),
                        in0=hview(g1a, ROW, 0, 2, 1),
                        in1=exview(0), op=Alu.mult)
                    nc.gpsimd.tensor_tensor(
                        out=hview(rhs[:], HF + 4, 2 * F, 2, 1),
                        in0=hview(g1a, ROW, 2 * F, 2, 1),
                        in1=exview(2), op=Alu.mult)
                    nc.scalar.copy(
                        bass.AP(rhs[:].tensor, rhs[:].offset + HF,
                                [rhs[:].ap[0], [HF + 4, g], [1, 4]]),
                        v3(ex[:], g, 4, 4, 1))
                    for k in range(g):
                        nc.tensor.matmul(
                            acc_ps[:],
                            lhsT=S[:, k * P:(k + 1) * P],
                            rhs=rhs[:, k * (HF + 4):(k + 1) * (HF + 4)],
                            start=(win_first[wi] == ci and k == 0),
                            stop=(win_last[wi] == ci and k == g - 1))

                    if win_last[wi] == ci:
                        # ---- window epilogue ----
                        nw = min(WIN, SH - wi * WIN)
                        den = fin.tile([P, HEADS], dt.float32, tag="den")
                        nc.vector.tensor_scalar(
                            den[:], acc_ps[:, HF:HF + 4], 1e-16, float(HEADS),
                            op0=Alu.add, op1=Alu.mult)
                        rec = fin.tile([P, HEADS], dt.float32, tag="rec")
                        nc.vector.reciprocal(rec[:], den[:])
                        prod = fin.tile([P, HF], dt.float32, tag="prod")
                        nc.vector.tensor_tensor(
                            out=v3(prod[:], HEADS, F, F, 1),
                            in0=v3(acc_ps[:, 0:HF], HEADS, F, F, 1),
                            in1=v3(rec[:], HEADS, 1, F, 0), op=Alu.mult)
                        gat = fin.tile([P, F], dt.float32, tag="gat")
                        nc.vector.tensor_reduce(
                            out=gat[:],
                            in_=v3(prod[:], F, 1, HEADS, F),
                            axis=mybir.AxisListType.X, op=Alu.add)
                        # prior
                        kgb = fin.tile([P, 32], dt.float32, tag="kgb")
                        if nw < P:
                            nc.gpsimd.memset(kgb[:], 0.0)
                        n0 = wi * WIN
                        nc.sync.dma_start(kgb[:nw, :], t_kg[n0:n0 + nw, :])
                        kgt_ps = prp.tile([32, P], dt.float32, space="PSUM",
                                          tag="kgt")
                        nc.tensor.transpose(out=kgt_ps[:], in_=kgb[:],
                                            identity=ident[:])
                        kgt_sb = fin.tile([32, P], dt.float32, tag="kgts")
                        nc.scalar.copy(kgt_sb[:], kgt_ps[:])
                        pr_ps = prp.tile([P, F], dt.float32, space="PSUM",
                                         tag="prps")
                        nc.tensor.matmul(pr_ps[:], lhsT=kgt_sb[:],
                                         rhs=wp_sb[:], start=True, stop=True)
                        # combine: (1-g)*(gat+bias) + g*(prior+bprior)
                        t1 = fin.tile([P, F], dt.float32, tag="t1")
                        nc.vector.tensor_tensor(out=t1[:], in0=gat[:],
                                                in1=bg_sb[:], op=Alu.add)
                        nc.vector.tensor_scalar_mul(t1[:], t1[:], g1m_sb[:, 0:1])
                        t2 = fin.tile([P, F], dt.float32, tag="t2")
                        nc.vector.tensor_tensor(out=t2[:], in0=pr_ps[:],
                                                in1=bp_sb[:], op=Alu.add)
                        nc.vector.tensor_scalar_mul(t2[:], t2[:], gate_sb[:, 0:1])
                        nc.vector.tensor_tensor(out=t1[:], in0=t1[:], in1=t2[:],
                                                op=Alu.add)
                        nc.sync.dma_start(t_out[n0:n0 + nw, :], t1[:nw, :])

    nc.compile()
    return nc


def _prepare(inputs):
    x = np.ascontiguousarray(np.asarray(inputs["x"], np.float32))
    edge_index = np.asarray(inputs["edge_index"])
    kg = np.ascontiguousarray(np.asarray(inputs["kg_onehot"], np.float32))
    wg = np.ascontiguousarray(np.asarray(inputs["W_gat"], np.float32))
    att_src = np.asarray(inputs["att_src"], np.float32)
    att_dst = np.asarray(inputs["att_dst"], np.float32)
    bias_gat = np.asarray(inputs["bias_gat"], np.float32)
    wp = np.ascontiguousarray(np.asarray(inputs["W_prior"], np.float32))
    b_prior = np.asarray(inputs["b_prior"], np.float32)
    gate = np.asarray(inputs["gate"], np.float32)

    chunks, B_wb, cblk, midx_p, tidx_p, dmod_c, dstf_c = _prep_edges(edge_index)

    iota_g = np.broadcast_to(
        np.tile(np.arange(P, dtype=np.float32), GMAX)[None, :],
        (P, GMAX * P)).copy()
    io16_g = np.broadcast_to(
        np.tile(np.arange(16, dtype=np.float32), GMAX)[None, :],
        (P, GMAX * 16)).copy()
    shared = {
        "x": x, "wg": wg, "wp": wp,
        "attsrc_b": np.broadcast_to(att_src.reshape(1, HF), (P, HF)).copy(),
        "attdst_b": np.broadcast_to(att_dst.reshape(1, HF), (P, HF)).copy(),
        "biasg_b": np.broadcast_to(bias_gat.reshape(1, F), (P, F)).copy(),
        "bprior_b": np.broadcast_to(b_prior.reshape(1, F), (P, F)).copy(),
        "gate_b": np.broadcast_to(gate.reshape(1, 1), (P, 1)).copy(),
        "iota_g": iota_g, "iota16_g": io16_g,
    }
    in_maps = []
    for c in range(NCORES):
        m = dict(shared)
        m["midx"] = midx_p[c]
        m["tidx"] = tidx_p[c]
        m["dmod"] = dmod_c[c]
        m["dstf"] = dstf_c[c]
        in_maps.append(m)

    for c in range(NCORES):
        in_maps[c]["kg"] = np.ascontiguousarray(kg[c * SH:(c + 1) * SH])

    nc = _build_nc(chunks, cblk)
    return nc, in_maps


def kernel(**inputs):
    nc, in_maps = _prepare(inputs)
    res = run_bass_kernel_spmd(nc, in_maps, core_ids=list(range(NCORES)))
    out = np.concatenate([res.results[c]["out"] for c in range(NCORES)], axis=0)
    return out.astype(np.float32)



# revision 6
# speedup vs baseline: 1.4934x; 1.4934x over previous
"""Trainium2 Bass kernel for ClimateConditionedGAT (GATConv + one-hot prior gate).

Strategy (8 NeuronCores, SPMD single NEFF):
  - Nodes sharded by destination: core c owns dst nodes [c*6250, (c+1)*6250).
  - Phase 0 (per core): project own dst shard, store a_dst in a local DRAM
    table t_Aloc (rows of 16 nodes, m-major, fp32).
  - Phase 1 (replicated): T[n] = [h interleaved (f,h) 256 | a_src 4] fp16,
    768B-stride rows, two 25001-row buckets (int16 gather indices), sentinel
    row per bucket (a_src = -6e4 => exp -> 0) for padding edges.
  - Phase 2: edges partitioned by (dst-window of 128, src bucket), sorted on
    host in bucket-major order; chunks of <=16 blocks (2048-row dma_gather of
    T by src + a_dst rows by local dst//16); DVE computes
    ex = exp(leaky(asrc+adst)) and rhs = [ex*h | ex] in fp16 (interleaved
    layout keeps all inner strides 1 for the 2x DVE mode); per 128-edge block
    a selection matrix S feeds a PE matmul accumulating into a per-window
    PSUM tile.  Bucket-0 partials spill to SBUF; bucket-1 epilogue combines,
    normalizes, averages heads, adds bias, and gates with the kg prior
    (host-transposed one-hot @ W_prior on PE).
Host does integer edge partitioning/packing plus dtype/layout casts only.
"""
import sys
import numpy as np

if "/opt/trn_rl_repo" not in sys.path:
    sys.path.insert(0, "/opt/trn_rl_repo")

from contextlib import ExitStack

import concourse.bass as bass
import concourse.bacc as bacc
import concourse.mybir as mybir
import concourse.tile as tile
from concourse.bass_utils import run_bass_kernel_spmd

P = 128
N = 50000
DIN = 128
HEADS = 4
F = 64
HF = HEADS * F          # 256
NCORES = 8
SH = N // NCORES        # 6250 dst nodes per core
ROW = 384               # T row stride in fp16 elems (768 B)
RWR = HF + HEADS        # 260 elems actually written per row (520 B)
BUCK_N = 25000          # real nodes per source bucket
BROWS = BUCK_N + 1      # bucket rows incl sentinel
SENT = BUCK_N           # bucket-local sentinel index
WIN = P                 # dst window size
NWIN = (SH + WIN - 1) // WIN    # 49 windows per core
AROWS = 400             # local a_dst table rows (6250/16 -> 391, padded)
GMAX = 16               # max 128-edge blocks per dma_gather
SB = 512                # phase-1 superblock (nodes per xT load / T write)
NEG_SLOPE = 0.2
NEG_BIG = -60000.0      # sentinel a_src (fp16-safe; exp underflows to 0)
dt = mybir.dt
Alu = mybir.AluOpType
Act = mybir.ActivationFunctionType
f16 = dt.float16


# ----------------------------------------------------------------------------
# host-side edge preparation (integer/layout work only)
# ----------------------------------------------------------------------------

def _pack_idx16(idx):
    """Pack int array -> [128, ceil(n/16)] int16 (wrap 16, replicate x8)."""
    n = len(idx)
    cols = (n + 15) // 16
    a = np.zeros((16, cols), np.int16)
    a[np.arange(n) % 16, np.arange(n) // 16] = idx.astype(np.int16)
    return np.tile(a, (8, 1))


def _prep_edges(edge_index):
    src = np.asarray(edge_index[0], dtype=np.int64)
    dst = np.asarray(edge_index[1], dtype=np.int64)
    loops = np.arange(N, dtype=np.int64)
    src = np.concatenate([src, loops])
    dst = np.concatenate([dst, loops])

    core = dst // SH
    w = (dst % SH) // WIN
    b = (src >= BUCK_N).astype(np.int64)

    # group id per edge: (core, b, w)  [bucket-major block order]
    gid = (core * 2 + b) * NWIN + w
    ngroups = NCORES * 2 * NWIN
    counts = np.bincount(gid, minlength=ngroups).reshape(NCORES, 2, NWIN)

    # uniform block counts across cores (shared NEFF)
    B_bw = np.ceil(counts.max(axis=0) / P).astype(np.int64)     # [2, NWIN]

    # block order: for b in (0,1): for w: B_bw[b,w] blocks
    block_b, block_w = [], []
    blk_first = np.zeros((2, NWIN), np.int64)   # first global blk of (b,w)
    blk_last = np.zeros((2, NWIN), np.int64)
    blk = 0
    for bi in range(2):
        for wi in range(NWIN):
            nb = int(B_bw[bi, wi])
            blk_first[bi, wi] = blk
            blk_last[bi, wi] = blk + nb - 1
            for _ in range(nb):
                block_b.append(bi)
                block_w.append(wi)
            blk += nb
    cblk = blk
    slots = cblk * P

    # chunks: runs of <= GMAX blocks within one bucket
    chunks = []                       # (bi, blk0, g)
    for bi in range(2):
        lo = blk_first[bi, 0]
        hi = blk_last[bi, NWIN - 1] + 1
        p0 = lo
        while p0 < hi:
            g = int(min(GMAX, hi - p0))
            chunks.append((bi, int(p0), g))
            p0 += g

    order = np.argsort(gid, kind="stable")
    src_s, dst_s = src[order], dst[order]
    grp_starts = np.zeros(ngroups + 1, np.int64)
    np.cumsum(np.bincount(gid[order], minlength=ngroups), out=grp_starts[1:])

    # slot offset of each (b,w) group inside the padded per-core layout
    slot_off = blk_first * P

    midx = np.full((NCORES, slots), SENT, np.int64)   # bucket-local src row
    tidx = np.zeros((NCORES, slots), np.int64)        # local dst // 16
    dmod = np.zeros((NCORES, slots), np.float32)      # local dst % 16
    dstf = np.zeros((NCORES, slots), np.float32)      # dst local to window

    for c in range(NCORES):
        for bi in range(2):
            for wi in range(NWIN):
                g = (c * 2 + bi) * NWIN + wi
                s0, s1 = grp_starts[g], grp_starts[g + 1]
                n = s1 - s0
                if n == 0:
                    continue
                o = slot_off[bi, wi]
                es, ed = src_s[s0:s1], dst_s[s0:s1]
                loc = ed - c * SH
                midx[c, o:o + n] = es - bi * BUCK_N
                tidx[c, o:o + n] = loc // 16
                dmod[c, o:o + n] = (loc % 16).astype(np.float32)
                dstf[c, o:o + n] = (loc - wi * WIN).astype(np.float32)

    midx_p = np.stack([_pack_idx16(midx[c]) for c in range(NCORES)])
    tidx_p = np.stack([_pack_idx16(tidx[c]) for c in range(NCORES)])
    # column-per-block layout: slot e -> [e % 128, e // 128]
    dmod_c = dmod.reshape(NCORES, cblk, P).transpose(0, 2, 1).astype(np.float16).copy()
    dstf_c = dstf.reshape(NCORES, cblk, P).transpose(0, 2, 1).astype(np.float16).copy()
    meta = dict(chunks=chunks, cblk=cblk, block_b=block_b, block_w=block_w,
                blk_first=blk_first, blk_last=blk_last)
    return meta, midx_p, tidx_p, dmod_c, dstf_c


# ----------------------------------------------------------------------------
# device program
# ----------------------------------------------------------------------------

def _build_nc(meta):
    cblk = meta["cblk"]
    chunks = meta["chunks"]
    block_w = meta["block_w"]
    blk_first = meta["blk_first"]
    blk_last = meta["blk_last"]

    nc = bacc.Bacc("TRN2", target_bir_lowering=False, debug=False)

    t_xT = nc.dram_tensor("xt16", [DIN, N], f16, kind="ExternalInput")
    t_xTw = nc.dram_tensor("xtw16", [DIN, SH], f16, kind="ExternalInput")
    t_kgT = nc.dram_tensor("kgt16", [32, NWIN * WIN], f16, kind="ExternalInput")
    t_wg = nc.dram_tensor("wg", [DIN, HF], dt.float32, kind="ExternalInput")
    t_asb = nc.dram_tensor("attsrc_b", [P, HF], dt.float32, kind="ExternalInput")
    t_adb = nc.dram_tensor("attdst_b", [P, HF], dt.float32, kind="ExternalInput")
    t_bg = nc.dram_tensor("biasg_b", [P, F], dt.float32, kind="ExternalInput")
    t_bp = nc.dram_tensor("bprior_b", [P, F], dt.float32, kind="ExternalInput")
    t_gate = nc.dram_tensor("gate_b", [P, 1], dt.float32, kind="ExternalInput")
    t_wp = nc.dram_tensor("wp", [32, F], dt.float32, kind="ExternalInput")
    t_iota = nc.dram_tensor("iota_g", [P, GMAX * P], f16, kind="ExternalInput")
    t_io16 = nc.dram_tensor("iota16_g", [P, GMAX * 16], f16, kind="ExternalInput")
    mcols = (cblk * P) // 16
    t_midx = nc.dram_tensor("midx", [P, mcols], dt.int16, kind="ExternalInput")
    t_tidx = nc.dram_tensor("tidx", [P, mcols], dt.int16, kind="ExternalInput")
    t_dmod = nc.dram_tensor("dmod", [P, cblk], f16, kind="ExternalInput")
    t_dstf = nc.dram_tensor("dstf", [P, cblk], f16, kind="ExternalInput")
    t_out = nc.dram_tensor("out", [SH, F], dt.float32, kind="ExternalOutput")

    t_T = [nc.dram_tensor(f"tbl{b}", [BROWS, ROW], f16) for b in range(2)]
    t_A = nc.dram_tensor("aloc", [AROWS, 16 * HEADS], dt.float32)

    with tile.TileContext(nc) as tc:
        with ExitStack() as octx:
            cp = octx.enter_context(tc.tile_pool(name="const", bufs=1))

            iota_sb = cp.tile([P, GMAX * P], f16)
            nc.sync.dma_start(iota_sb[:], t_iota[:])
            io16_sb = cp.tile([P, GMAX * 16], f16)
            nc.sync.dma_start(io16_sb[:], t_io16[:])
            wp_sb = cp.tile([32, F], dt.float32)
            nc.sync.dma_start(wp_sb[:], t_wp[:])
            wp16 = cp.tile([32, F], f16)
            nc.vector.tensor_scalar_add(wp16[:], wp_sb[:], 0.0)
            asb = cp.tile([P, HF], dt.float32)
            nc.sync.dma_start(asb[:], t_asb[:])
            adb = cp.tile([P, HF], dt.float32)
            nc.sync.dma_start(adb[:], t_adb[:])
            bg_sb = cp.tile([P, F], dt.float32)
            nc.sync.dma_start(bg_sb[:], t_bg[:])
            bp_sb = cp.tile([P, F], dt.float32)
            nc.sync.dma_start(bp_sb[:], t_bp[:])
            gate_sb = cp.tile([P, 1], dt.float32)
            nc.sync.dma_start(gate_sb[:], t_gate[:])
            g1m_sb = cp.tile([P, 1], dt.float32)
            nc.vector.tensor_scalar(g1m_sb[:], gate_sb[:], -1.0, 1.0,
                                    op0=Alu.mult, op1=Alu.add)
            midx_sb = cp.tile([P, mcols], dt.int16)
            nc.sync.dma_start(midx_sb[:], t_midx[:])
            tidx_sb = cp.tile([P, mcols], dt.int16)
            nc.sync.dma_start(tidx_sb[:], t_tidx[:])
            dmod_sb = cp.tile([P, cblk], f16)
            nc.sync.dma_start(dmod_sb[:], t_dmod[:])
            dstf_sb = cp.tile([P, cblk], f16)
            nc.sync.dma_start(dstf_sb[:], t_dstf[:])
            kgT_sb = cp.tile([32, NWIN * WIN], f16)
            nc.sync.dma_start(kgT_sb[:], t_kgT[:])
            # bucket-0 window accumulator spill
            wacc = cp.tile([P, NWIN, RWR], dt.float32)

            # wext16 = [W interleaved (f,h) | wsrc(4) | wdst(4)] in fp16
            # wsrc_j = W[:, head j] . att_src[j]
            wg_sb = cp.tile([P, HF], dt.float32)
            nc.sync.dma_start(wg_sb[:], t_wg[:])
            wext = cp.tile([P, HF + 8], f16)
            nc.scalar.copy(wext[:, 0:HF], wg_sb[:])
            with tc.tile_pool(name="wtmp", bufs=2) as wtp:
                for j in range(HEADS):
                    wslice = bass.AP(wg_sb[:].tensor, wg_sb[:].offset + j,
                                     [wg_sb[:].ap[0], [HEADS, F]])
                    for k, att in enumerate((asb, adb)):
                        tmp = wtp.tile([P, F], dt.float32, tag="wtmp")
                        nc.vector.tensor_tensor(
                            out=tmp[:], in0=wslice,
                            in1=att[:, j * F:(j + 1) * F], op=Alu.mult)
                        tred = wtp.tile([P, 1], dt.float32, tag="tred")
                        nc.vector.tensor_reduce(
                            out=tred[:], in_=tmp[:],
                            axis=mybir.AxisListType.X, op=Alu.add)
                        nc.vector.tensor_scalar_add(
                            wext[:, HF + 4 * k + j:HF + 4 * k + j + 1],
                            tred[:], 0.0)

            # ---------------- phase 0 + 1: projections ----------------
            with tc.tile_pool(name="p1sb", bufs=3) as p1, \
                 tc.tile_pool(name="p1ps", bufs=3, space="PSUM") as pp1, \
                 tc.tile_pool(name="p1st", bufs=3) as pst:

                # phase 0: own-shard projection -> local a_dst table
                nsb0 = (SH + SB - 1) // SB
                awst = cp.tile([P, nsb0 * 4, HEADS], dt.float32)
                bi0 = 0
                for sb in range(nsb0):
                    ns = sb * SB
                    nn = min(SB, SH - ns)
                    xt = p1.tile([P, SB], f16, tag="xt")
                    if nn < SB:
                        nc.gpsimd.memset(xt[:], 0.0)
                    nc.sync.dma_start(xt[:, 0:nn], t_xTw[:, ns:ns + nn])
                    nbl = (nn + P - 1) // P
                    for k in range(nbl):
                        h_ps = pp1.tile([P, HF + 8], dt.float32, space="PSUM",
                                        tag="hps")
                        nc.tensor.matmul(h_ps[:], lhsT=xt[:, k * P:(k + 1) * P],
                                         rhs=wext[:], start=True, stop=True)
                        nc.vector.tensor_scalar_add(awst[:, bi0, :],
                                                    h_ps[:, HF + 4:HF + 8],
                                                    0.0)
                        bi0 += 1
                # one DMA for the whole local a_dst table
                adst_dst = bass.AP(t_A[:, :].tensor, 0,
                                   [[16 * HEADS, 8], [HEADS, 16],
                                    [8 * 16 * HEADS, bi0], [1, HEADS]])
                nc.sync.dma_start(adst_dst, awst[:, 0:bi0, :])

                # phase 1: full projection tables (bucket 0 then bucket 1)
                for bkt in range(2):
                    nsb = (BUCK_N + SB - 1) // SB
                    for sb in range(nsb):
                        ns = sb * SB
                        nn = min(SB, BUCK_N - ns)
                        xt = p1.tile([P, SB], f16, tag="xt")
                        if nn < SB:
                            nc.gpsimd.memset(xt[:], 0.0)
                        nc.sync.dma_start(xt[:, 0:nn],
                                          t_xT[:, bkt * BUCK_N + ns:
                                               bkt * BUCK_N + ns + nn])
                        nfull = nn // P
                        stage = pst.tile([P, SB // P, RWR], f16, tag="stage")
                        for k in range((nn + P - 1) // P):
                            h_ps = pp1.tile([P, HF + 8], dt.float32,
                                            space="PSUM", tag="hps")
                            nc.tensor.matmul(h_ps[:],
                                             lhsT=xt[:, k * P:(k + 1) * P],
                                             rhs=wext[:], start=True, stop=True)
                            eng = nc.vector if k % 2 == 0 else nc.scalar
                            if k < nfull:
                                if eng is nc.vector:
                                    nc.vector.tensor_scalar_add(
                                        stage[:, k, :], h_ps[:, 0:RWR], 0.0)
                                else:
                                    nc.scalar.copy(stage[:, k, :],
                                                   h_ps[:, 0:RWR])
                            else:
                                # partial tail block: write rows directly
                                tl = p1.tile([P, RWR], f16, tag="tail")
                                nc.scalar.copy(tl[:], h_ps[:, 0:RWR])
                                nr = nn - nfull * P
                                nc.sync.dma_start(
                                    t_T[bkt][ns + nfull * P:
                                             ns + nfull * P + nr, 0:RWR],
                                    tl[:nr, :])
                        if nfull > 0:
                            dst = t_T[bkt][ns:ns + nfull * P, 0:RWR] \
                                .rearrange("(b p) c -> p b c", p=P)
                            nc.sync.dma_start(dst, stage[:, 0:nfull, :])
                    # sentinel row
                    sent = p1.tile([1, RWR], f16, tag="sent")
                    nc.gpsimd.memset(sent[:], 0.0)
                    nc.gpsimd.memset(sent[:, HF:RWR], NEG_BIG)
                    nc.sync.dma_start(t_T[bkt][SENT:SENT + 1, 0:RWR], sent[:])

                # ---------------- phase 2: gather / scatter ----------------
                with tc.tile_pool(name="g1p", bufs=2) as g1p, \
                     tc.tile_pool(name="g2p", bufs=2) as g2p, \
                     tc.tile_pool(name="wk", bufs=2) as wk, \
                     tc.tile_pool(name="sp", bufs=2) as sp, \
                     tc.tile_pool(name="accp", bufs=3, space="PSUM") as accp, \
                     tc.tile_pool(name="prp", bufs=2, space="PSUM") as prp, \
                     tc.tile_pool(name="fin", bufs=3) as fin:

                    def v3(t, d1, s1, d2, s2):
                        return bass.AP(t.tensor, t.offset,
                                       [t.ap[0], [s1, d1], [s2, d2]])

                    def v4(t, d1, s1, d2, s2, d3, s3):
                        return bass.AP(t.tensor, t.offset,
                                       [t.ap[0], [s1, d1], [s2, d2], [s3, d3]])

                    acc_ps = None
                    for (bi, blk0, g) in chunks:
                        nidx = g * P
                        ic0 = blk0 * 8
                        icn = nidx // 16
                        g1 = g1p.tile([P, GMAX, ROW], f16, tag="g1")
                        nc.gpsimd.dma_gather(
                            g1[:, 0:g, :], t_T[bi][:, :],
                            midx_sb[:, ic0:ic0 + icn], nidx, nidx, ROW)
                        g2 = g2p.tile([P, GMAX, 64], dt.float32, tag="g2")
                        nc.gpsimd.dma_gather(
                            g2[:, 0:g, :], t_A[:, :],
                            tidx_sb[:, ic0:ic0 + icn], nidx, nidx, 64)
                        g1a = g1[:, 0:g, :]
                        g2a = g2[:, 0:g, :]

                        # mask[e, m] = (dst%16 == m)   [P, g, 16]
                        mask = wk.tile([P, GMAX * 16], f16, tag="mask")
                        nc.vector.tensor_tensor(
                            out=v3(mask[:], g, 16, 16, 1),
                            in0=v3(dmod_sb[:, blk0:blk0 + g], g, 1, 16, 0),
                            in1=v3(io16_sb[:, 0:g * 16], g, 16, 16, 1),
                            op=Alu.is_equal)
                        # adst_sel[e, j] = sum_m mask[e,m] * A_g[e, m*4+j]
                        selt = wk.tile([P, GMAX * 64], f16, tag="selt")
                        nc.vector.tensor_tensor(
                            out=v4(selt[:], g, 64, 16, 4, 4, 1),
                            in0=v4(mask[:], g, 16, 16, 1, 4, 0),
                            in1=v4(g2a, g, 64, 16, 4, 4, 1),
                            op=Alu.mult)
                        adst = wk.tile([P, GMAX * 4], dt.float32, tag="adst")
                        nc.vector.tensor_reduce(
                            out=v3(adst[:], g, 4, 4, 1),
                            in_=v4(selt[:], g, 64, 4, 1, 16, 4),
                            axis=mybir.AxisListType.X, op=Alu.add)
                        # alpha = leaky(asrc + adst); ex = exp(alpha)
                        alpha = wk.tile([P, GMAX * 4], f16, tag="alpha")
                        nc.vector.tensor_tensor(
                            out=v3(alpha[:], g, 4, 4, 1),
                            in0=v3(bass.AP(g1a.tensor, g1a.offset + HF,
                                           [g1a.ap[0]]), g, ROW, 4, 1),
                            in1=v3(adst[:], g, 4, 4, 1), op=Alu.add)
                        nc.vector.scalar_tensor_tensor(
                            out=alpha[:, 0:g * 4], in0=alpha[:, 0:g * 4],
                            scalar=NEG_SLOPE, in1=alpha[:, 0:g * 4],
                            op0=Alu.mult, op1=Alu.max)
                        ex = wk.tile([P, GMAX * 4], f16, tag="ex")
                        nc.scalar.activation(ex[:, 0:g * 4], alpha[:, 0:g * 4],
                                             Act.Exp)
                        # S[e, d] = (dstf == d)   [P, g, 128]
                        S = sp.tile([P, GMAX * P], f16, tag="S")
                        nc.vector.tensor_tensor(
                            out=v3(S[:], g, P, P, 1),
                            in0=v3(dstf_sb[:, blk0:blk0 + g], g, 1, P, 0),
                            in1=v3(iota_sb[:, 0:g * P], g, P, P, 1),
                            op=Alu.is_equal)
                        # rhs = [ex*h (256, (f,h) interleaved) | ex (4)]
                        rhs = wk.tile([P, GMAX * RWR], f16, tag="rhs")
                        nc.vector.tensor_tensor(
                            out=v4(rhs[:], g, RWR, F, HEADS, HEADS, 1),
                            in0=v4(g1a, g, ROW, F, HEADS, HEADS, 1),
                            in1=v4(ex[:], g, 4, F, 0, HEADS, 1),
                            op=Alu.mult)
                        nc.scalar.copy(
                            bass.AP(rhs[:].tensor, rhs[:].offset + HF,
                                    [rhs[:].ap[0], [RWR, g], [1, 4]]),
                            v3(ex[:], g, 4, 4, 1))

                        for k in range(g):
                            blk = blk0 + k
                            wi = block_w[blk]
                            first = blk == blk_first[bi, wi]
                            last = blk == blk_last[bi, wi]
                            if first:
                                acc_ps = accp.tile([P, RWR], dt.float32,
                                                   space="PSUM", tag="acc")
                            nc.tensor.matmul(
                                acc_ps[:],
                                lhsT=S[:, k * P:(k + 1) * P],
                                rhs=rhs[:, k * RWR:(k + 1) * RWR],
                                start=first, stop=last)
                            if not last:
                                continue
                            if bi == 0:
                                # spill bucket-0 partials to SBUF
                                if wi % 2 == 0:
                                    nc.vector.tensor_scalar_add(
                                        wacc[:, wi, :], acc_ps[:], 0.0)
                                else:
                                    nc.scalar.copy(wacc[:, wi, :], acc_ps[:])
                                continue
                            # ---- window epilogue (bucket 1 done) ----
                            nw = min(WIN, SH - wi * WIN)
                            tot = fin.tile([P, RWR], dt.float32, tag="tot")
                            nc.vector.tensor_tensor(
                                out=tot[:], in0=acc_ps[:], in1=wacc[:, wi, :],
                                op=Alu.add)
                            den = fin.tile([P, HEADS], dt.float32, tag="den")
                            nc.vector.tensor_scalar(
                                den[:], tot[:, HF:RWR], 1e-16, float(HEADS),
                                op0=Alu.add, op1=Alu.mult)
                            rec = fin.tile([P, HEADS], dt.float32, tag="rec")
                            nc.vector.reciprocal(rec[:], den[:])
                            prod = fin.tile([P, HF], dt.float32, tag="prod")
                            nc.vector.tensor_tensor(
                                out=v3(prod[:], F, HEADS, HEADS, 1),
                                in0=v3(tot[:, 0:HF], F, HEADS, HEADS, 1),
                                in1=v3(rec[:], F, 0, HEADS, 1), op=Alu.mult)
                            gat = fin.tile([P, F], dt.float32, tag="gat")
                            nc.vector.tensor_reduce(
                                out=gat[:],
                                in_=v3(prod[:], F, HEADS, HEADS, 1),
                                axis=mybir.AxisListType.X, op=Alu.add)
                            # prior = kg_onehot @ W_prior
                            pr_ps = prp.tile([P, F], dt.float32, space="PSUM",
                                             tag="prps")
                            nc.tensor.matmul(
                                pr_ps[:],
                                lhsT=kgT_sb[:, wi * WIN:(wi + 1) * WIN],
                                rhs=wp16[:], start=True, stop=True)
                            # combine: (1-g)*(gat+bias) + g*(prior+bprior)
                            t1 = fin.tile([P, F], dt.float32, tag="t1")
                            nc.vector.tensor_tensor(out=t1[:], in0=gat[:],
                                                    in1=bg_sb[:], op=Alu.add)
                            nc.vector.tensor_scalar_mul(t1[:], t1[:],
                                                        g1m_sb[:, 0:1])
                            t2 = fin.tile([P, F], dt.float32, tag="t2")
                            nc.vector.tensor_tensor(out=t2[:], in0=pr_ps[:],
                                                    in1=bp_sb[:], op=Alu.add)
                            nc.vector.tensor_scalar_mul(t2[:], t2[:],
                                                        gate_sb[:, 0:1])
                            nc.vector.tensor_tensor(out=t1[:], in0=t1[:],
                                                    in1=t2[:], op=Alu.add)
                            n0 = wi * WIN
                            nc.sync.dma_start(t_out[n0:n0 + nw, :], t1[:nw, :])

    nc.compile()
    return nc


def _prepare(inputs):
    x = np.asarray(inputs["x"], np.float32)
    edge_index = np.asarray(inputs["edge_index"])
    kg = np.asarray(inputs["kg_onehot"], np.float32)
    wg = np.asarray(inputs["W_gat"], np.float32)
    att_src = np.asarray(inputs["att_src"], np.float32)
    att_dst = np.asarray(inputs["att_dst"], np.float32)
    bias_gat = np.asarray(inputs["bias_gat"], np.float32)
    wp = np.ascontiguousarray(np.asarray(inputs["W_prior"], np.float32))
    b_prior = np.asarray(inputs["b_prior"], np.float32)
    gate = np.asarray(inputs["gate"], np.float32)

    meta, midx_p, tidx_p, dmod_c, dstf_c = _prep_edges(edge_index)

    xT16 = np.ascontiguousarray(x.T.astype(np.float16))       # [128, N]
    # W_gat with (f, h)-interleaved output columns
    wg_il = np.ascontiguousarray(
        wg.reshape(DIN, HEADS, F).transpose(0, 2, 1).reshape(DIN, HF))
    as_il = np.broadcast_to(att_src.reshape(1, HF), (P, HF)).copy()
    ad_il = np.broadcast_to(att_dst.reshape(1, HF), (P, HF)).copy()

    iota_g = np.broadcast_to(
        np.tile(np.arange(P, dtype=np.float16), GMAX)[None, :],
        (P, GMAX * P)).copy()
    io16_g = np.broadcast_to(
        np.tile(np.arange(16, dtype=np.float16), GMAX)[None, :],
        (P, GMAX * 16)).copy()
    shared = {
        "xt16": xT16, "wg": wg_il, "wp": wp,
        "attsrc_b": as_il, "attdst_b": ad_il,
        "biasg_b": np.broadcast_to(bias_gat.reshape(1, F), (P, F)).copy(),
        "bprior_b": np.broadcast_to(b_prior.reshape(1, F), (P, F)).copy(),
        "gate_b": np.broadcast_to(gate.reshape(1, 1), (P, 1)).copy(),
        "iota_g": iota_g, "iota16_g": io16_g,
    }
    in_maps = []
    for c in range(NCORES):
        m = dict(shared)
        m["midx"] = midx_p[c]
        m["tidx"] = tidx_p[c]
        m["dmod"] = dmod_c[c]
        m["dstf"] = dstf_c[c]
        m["xtw16"] = np.ascontiguousarray(xT16[:, c * SH:(c + 1) * SH])
        kgt = np.zeros((32, NWIN * WIN), np.float16)
        kgt[:, 0:SH] = kg[c * SH:(c + 1) * SH].T.astype(np.float16)
        m["kgt16"] = kgt
        in_maps.append(m)

    nc = _build_nc(meta)
    return nc, in_maps


def kernel(**inputs):
    nc, in_maps = _prepare(inputs)
    res = run_bass_kernel_spmd(nc, in_maps, core_ids=list(range(NCORES)))
    out = np.concatenate([res.results[c]["out"] for c in range(NCORES)], axis=0)
    return out.astype(np.float32)
